# revision 6
# baseline (speedup 1.0000x reference)
"""Trainium2 Bass kernel for the BMP loss (nn_BMPLoss_24670292148307). V4.

Data-parallel over 8 NeuronCores; host combines per-core partial sums.

V4 redesign vs the 34us V3 (critical-path + engine-rebalance):
  - vertex L1 via the identity sum|a-b| = 2*sum(max(a,b)) - sum(a+b):
    ONE DVE tensor_tensor_reduce (op0=max, scale=2, chained accum) per
    chunk replaces the old add(DVE)+abs-accumulate(ACT) pair; the
    sum(a+b) correction runs on the otherwise-idle TensorE as ones^T
    matmuls accumulating into one PSUM bank, drained by a single ACT
    copy-accumulate.  ACT's ~5.7us of vertex Abs work disappears.
  - va/gt ship interleaved per chunk in ONE dram tensor (one DMA trigger
    per chunk, all on the SP HWDGE queue; blk/cst/out on the ACT queue) —
    DIRECT2D triggers cost ~0.6us of sequencer time each.
  - 4 chunks sized [1792,1792,1536,512] (2w % 512 == 0 so every matmul
    block is exactly 512 wide; small last chunk shrinks the post-DMA
    tail).
  - Chebyshev outer-root evaluation in power basis: build [r..r^9] in 4
    DVE ops + one mul + one reduce (depth ~0.5us) instead of the 10-step
    Horner chain (2.1us serial).
  - A^2 computed right after A (off critical path); det(K) branch filled
    into the p=sqrt() ACT wait; scl coefficient (pv2*v1i/3) precomputed
    before the s=sqrt(lam) wait so only 4 small ops remain after it.
  - pose/betas/shape masks folded into the host pack (rows pre-zeroed);
    mask multiply ops and comp memset dropped (host sums only the rows
    each accumulator actually writes).
  - ~10 warm matmuls on scratch ramp the PE clock (1.2->2.4GHz) before
    the real column-sum matmuls arrive.
All Procrustes numerics are bit-compatible with V3 (same polynomial,
same op semantics, fp reassociation only).
"""
import numpy as np
from contextlib import ExitStack

import concourse.bass as bass
import concourse.bacc as bacc
import concourse.tile as tile
import concourse.mybir as mybir
from concourse.bass_utils import run_bass_kernel_spmd

f32 = mybir.dt.float32
bf16 = mybir.dt.bfloat16
AF = mybir.ActivationFunctionType
OP = mybir.AluOpType
AX = mybir.AxisListType

B_PER_CORE = 64
N_CORES = 8
J = 24
VERT_F = 20670           # real floats per sample (6890*3)
F_PACK = 5632            # padded stream cols per tensor (22*256)
PACK_CAP = 34            # vertex slots per core (128*5632/20670 = 34.8)
W_CHUNKS = [1792, 1792, 1536, 512]
EPS = 1e-8
TINY = 1e-30
RCLAMP = 0.99995

# blk (bf16) column map
PG6 = slice(0, 144)      # (c,n): rows 0-2 pj xyz, 3-5 gt3 xyz, joint-minor
CONF3 = slice(144, 168)
CAM = slice(168, 171)
G2 = slice(171, 219)     # (c,n), pre-shifted by -256, /1000
CONF2 = slice(219, 243)
RP = slice(243, 459)     # pre-masked by has_smpl
RG = slice(459, 675)     # pre-masked
PB = slice(675, 685)     # pre-masked
GS = slice(685, 695)     # pre-masked
BLK_COLS = 695

# cst (f32) column map
PC = slice(0, 18)        # cols 0-8 = P1C[1..9], 9-17 = P3C[1..9]
C0 = slice(18, 20)       # [P1C[0], P3C[0]]
EYE9 = slice(20, 29)
EYE3 = slice(29, 38)     # eye/3
RCL = slice(38, 39)      # RCLAMP
CST_COLS = 40

P1C = [0.8649274597522203, 0.17578197434414333, -0.002087134697444787,
       -0.1271791091353304, -0.3070988770461487, 0.6789215326112841,
       0.5727490378285598, -1.068537975408937, -0.3683220235409602,
       0.5818562170395759]
P3C = [-0.8649274597522203, 0.17578197434414353, 0.002087134697442622,
       -0.1271791091353331, 0.3070988770461617, 0.6789215326112932,
       -0.5727490378285826, -1.068537975408948, 0.3683220235409723,
       0.58185621703958]

N_WARM_MM = 10
USE_MM = True
USE_TTR = True
USE_WARM_MM = True


def _cst_array() -> np.ndarray:
    c = np.zeros((B_PER_CORE, CST_COLS), np.float32)
    for t in range(9):
        c[:, t] = np.float32(P1C[t + 1])
        c[:, 9 + t] = np.float32(P3C[t + 1])
    c[:, 18] = np.float32(P1C[0])
    c[:, 19] = np.float32(P3C[0])
    eye = np.eye(3, dtype=np.float32).reshape(9)
    c[:, EYE9] = eye
    c[:, EYE3] = eye / 3.0
    c[:, RCL] = RCLAMP
    return c


def build_program():
    nc = bacc.Bacc("TRN2", target_bir_lowering=False, debug=False,
                   num_devices=N_CORES)
    P = B_PER_CORE

    cst_d = nc.dram_tensor("cst", [P, CST_COLS], f32, kind="ExternalInput")
    blk_d = nc.dram_tensor("blk", [P, BLK_COLS], bf16, kind="ExternalInput")
    vab_d = nc.dram_tensor("vab", [128, 2 * F_PACK], bf16,
                           kind="ExternalInput")
    out_d = nc.dram_tensor("out", [128, 8], f32, kind="ExternalOutput")

    with tile.TileContext(nc) as tc, ExitStack() as ctx:
        V = nc.vector
        A = nc.scalar
        G = nc.gpsimd
        SP = nc.sync
        T = nc.tensor
        sg = ctx.enter_context(tc.tile_pool(name="singles", bufs=1))
        vp = ctx.enter_context(tc.tile_pool(name="vp", bufs=2))
        pp_ = ctx.enter_context(tc.tile_pool(name="ps", bufs=1, space="PSUM"))

        def S(shape, name, dtype=f32):
            return sg.tile(list(shape), dtype, name=name)

        comp = S([128, 8], "comp")

        # first ACT op is a Sqrt so the table loader picks the sqrt set once
        warm = S([1, 1], "warm")
        G.memset(warm[:, :], 1.0)
        warm2 = S([1, 1], "warm2")
        A.activation(warm2[:, :], warm[:, :], AF.Sqrt)

        ones = S([128, 1], "ones", bf16)
        G.memset(ones[:, :], 1.0)
        wscr = S([128, 512], "wscr", bf16)
        G.memset(wscr[:, :], 0.0)

        # ---------------- input DMAs ----------------------------------------
        blk_t = S([P, BLK_COLS], "blk_t", bf16)
        A.dma_start(blk_t[:, :], blk_d[:, :])
        cst_t = S([P, CST_COLS], "cst_t")
        A.dma_start(cst_t[:, :], cst_d[:, :])
        vab_ts = []
        off = 0
        for c, w in enumerate(W_CHUNKS):
            vt = sg.tile([128, 2 * w], bf16, name=f"vab{c}")
            SP.dma_start(vt[:, :], vab_d[:, 2 * off:2 * off + 2 * w])
            vab_ts.append(vt)
            off += w

        # PE clock warm-up: ones^T @ zeros into a scratch PSUM bank
        pswarm = pp_.tile([1, 512], f32, name="pswarm")
        if USE_MM and USE_WARM_MM:
            for i in range(N_WARM_MM):
                T.matmul(pswarm[:, :], ones[:, :], wscr[:, :],
                         start=True, stop=True)
        psacc = pp_.tile([1, 512], f32, name="psacc")

        pg6 = blk_t[:, PG6]
        eye9 = cst_t[:, EYE9]
        eye3 = cst_t[:, EYE3]
        t1 = S([P, 1], "t1")

        # ================ Procrustes chain (DVE) ================
        musum = S([P, 6], "musum")
        V.tensor_reduce(musum[:, :], pg6.rearrange("p (c n) -> p c n", n=J),
                        axis=AX.X, op=OP.add)
        mu6 = S([P, 6], "mu6")
        V.tensor_single_scalar(mu6[:, :], musum[:, :], 1.0 / J, OP.mult)
        Xn = S([P, 144], "Xn")     # (mu - pg6): negated centered coords
        V.tensor_sub(Xn[:, :].rearrange("p (c n) -> p c n", n=J),
                     mu6[:, :].unsqueeze(2).broadcast_to([P, 6, J]),
                     pg6.rearrange("p (c n) -> p c n", n=J))
        X1n = Xn[:, 0:72]
        X2n = Xn[:, 72:144]
        var1 = S([P, 1], "var1")
        vscr = S([P, 72], "vscr")
        A.activation(vscr[:, :], X1n, AF.Square, accum_out=var1[:, :])
        V.tensor_scalar(t1[:, :], blk_t[:, CAM][:, 0:1], 512.0, EPS,
                        OP.mult, OP.add)
        rt1 = S([P, 1], "rt1")
        V.reciprocal(rt1[:, :], t1[:, :])

        # kp2d prep front-loaded on Pool so rzt slots into the chain early
        depth = S([P, 1], "depth")
        G.tensor_single_scalar(depth[:, :], rt1[:, :], 2000.0, OP.mult)
        pxy = S([P, 48], "pxy", bf16)
        G.tensor_add(pxy[:, :].rearrange("p (c n) -> p c n", n=J),
                     blk_t[:, PG6].rearrange("p (c n) -> p c n", n=J)[:, 0:2],
                     blk_t[:, CAM][:, 1:3].unsqueeze(2).broadcast_to([P, 2, J]))
        pzt = S([P, J], "pzt")
        G.tensor_add(pzt[:, :], blk_t[:, 48:72],
                     depth[:, :].broadcast_to([P, J]))

        # K = X1 X2^T
        kq = S([P, 216], "kq")
        V.tensor_mul(
            kq[:, :].rearrange("p (i j n) -> p i j n", i=3, j=3),
            X1n.rearrange("p (i n) -> p i n", i=3)
                .unsqueeze(2).broadcast_to([P, 3, 3, J]),
            X2n.rearrange("p (j n) -> p j n", j=3)
                .unsqueeze(1).broadcast_to([P, 3, 3, J]))
        K9 = S([P, 9], "K9")
        V.tensor_reduce(K9[:, :], kq[:, :].rearrange(
            "p (i j n) -> p i j n", i=3, j=3), axis=AX.X, op=OP.add)

        # det(K) on DVE (feeds detA for r, and the sign)
        dQ = S([P, 9], "dQ")
        V.tensor_mul(
            dQ[:, :].rearrange("p (a b) -> p a b", a=3),
            K9[:, 3:6].unsqueeze(2).broadcast_to([P, 3, 3]),
            K9[:, 6:9].unsqueeze(1).broadcast_to([P, 3, 3]))
        dD = S([P, 9], "dD")
        V.tensor_sub(dD[:, :].rearrange("p (a b) -> p a b", a=3),
                     dQ[:, :].rearrange("p (a b) -> p a b", a=3),
                     dQ[:, :].rearrange("p (b a) -> p a b", b=3))
        du1 = S([P, 2], "du1")
        V.tensor_mul(du1[:, :], K9[:, 0:2], dD[:, 5:7])
        du2 = S([P, 1], "du2")
        V.tensor_mul(du2[:, :], K9[:, 2:3], dD[:, 1:2])
        du1r = S([P, 1], "du1r")
        V.tensor_reduce(du1r[:, :], du1[:, :], axis=AX.X, op=OP.add)
        detK = S([P, 1], "detK")
        V.tensor_add(detK[:, :], du1r[:, :], du2[:, :])
        detA = S([P, 1], "detA")
        V.tensor_mul(detA[:, :], detK[:, :], detK[:, :])
        sg0 = S([P, 1], "sg0")
        V.tensor_single_scalar(sg0[:, :], detK[:, :], 0.0, OP.is_ge)
        sgn = S([P, 1], "sgn")
        V.tensor_scalar(sgn[:, :], sg0[:, :], 2.0, -1.0, OP.mult, OP.add)

        # A = K^T K
        aq = S([P, 27], "aq")
        V.tensor_mul(
            aq[:, :].rearrange("p (i j k) -> p i j k", i=3, j=3),
            K9[:, :].rearrange("p (k i) -> p i k", k=3)
                .unsqueeze(2).broadcast_to([P, 3, 3, 3]),
            K9[:, :].rearrange("p (k j) -> p j k", k=3)
                .unsqueeze(1).broadcast_to([P, 3, 3, 3]))
        A9 = S([P, 9], "A9")
        V.tensor_reduce(A9[:, :], aq[:, :].rearrange(
            "p (i j k) -> p i j k", i=3, j=3), axis=AX.X, op=OP.add)
        # A^2 right after A: needed only at W, but free to do now
        a2q = S([P, 27], "a2q")
        V.tensor_mul(
            a2q[:, :].rearrange("p (i j k) -> p i j k", i=3, j=3),
            A9[:, :].rearrange("p (i k) -> p i k", i=3)
                .unsqueeze(2).broadcast_to([P, 3, 3, 3]),
            A9[:, :].rearrange("p (k j) -> p j k", k=3)
                .unsqueeze(1).broadcast_to([P, 3, 3, 3]))
        A29 = S([P, 9], "A29")
        V.tensor_reduce(A29[:, :], a2q[:, :].rearrange(
            "p (i j k) -> p i j k", i=3, j=3), axis=AX.X, op=OP.add)

        qsum = S([P, 1], "qsum")
        V.tensor_reduce(qsum[:, :], A9[:, 0:9:4], axis=AX.X, op=OP.add)
        q3rd = S([P, 1], "q3rd")
        V.tensor_single_scalar(q3rd[:, :], qsum[:, :], 1.0 / 3.0, OP.mult)
        q2 = S([P, 1], "q2")
        V.tensor_mul(q2[:, :], q3rd[:, :], q3rd[:, :])
        q3 = S([P, 1], "q3")
        V.tensor_mul(q3[:, :], q2[:, :], q3rd[:, :])
        nqsum = S([P, 1], "nqsum")
        V.tensor_single_scalar(nqsum[:, :], qsum[:, :], -1.0, OP.mult)
        aqn = S([P, 9], "aqn")
        V.scalar_tensor_tensor(aqn[:, :], eye3, qsum[:, :], A9[:, :],
                               OP.mult, OP.subtract)
        pscr = S([P, 9], "pscr")
        V.tensor_mul(pscr[:, :], aqn[:, :], aqn[:, :])
        p2r = S([P, 1], "p2r")
        V.tensor_reduce(p2r[:, :], pscr[:, :], axis=AX.X, op=OP.add)
        p2g = S([P, 1], "p2g")
        V.tensor_scalar(p2g[:, :], p2r[:, :], 1.0 / 6.0, TINY,
                        OP.mult, OP.max)
        pp = S([P, 1], "pp")
        A.activation(pp[:, :], p2g[:, :], AF.Sqrt)
        tp = S([P, 1], "tp")
        G.tensor_single_scalar(tp[:, :], pp[:, :], 2.0, OP.mult)

        # z = detA - q^3 + 3 q p^2 (fills the sqrt wait)
        zu = S([P, 1], "zu")
        V.tensor_mul(zu[:, :], q3rd[:, :], p2g[:, :])
        zv = S([P, 1], "zv")
        V.scalar_tensor_tensor(zv[:, :], zu[:, :], 3.0, q3[:, :],
                               OP.mult, OP.subtract)
        zz = S([P, 1], "zz")
        V.tensor_add(zz[:, :], detA[:, :], zv[:, :])

        # ---------------- kp3d (Pool prep, ACT accumulate) ------------------
        pd = S([P, 72], "pd", bf16)
        G.tensor_sub(pd[:, :], blk_t[:, 0:72], blk_t[:, 72:144])
        pdr = pd[:, :].rearrange("p (c n) -> p c n", n=J)
        pel = S([P, 3], "pel", bf16)
        G.tensor_add(pel[:, :], pdr[:, :, 2].squeeze(), pdr[:, :, 3].squeeze())
        pel2 = S([P, 3], "pel2", bf16)
        G.tensor_single_scalar(pel2[:, :], pel[:, :], 0.5, OP.mult)
        d3n = S([P, 72], "d3n", bf16)
        G.tensor_sub(d3n[:, :].rearrange("p (c n) -> p c n", n=J),
                     pdr, pel2[:, :].unsqueeze(2).broadcast_to([P, 3, J]))
        u3d = S([P, 72], "u3d", bf16)
        G.tensor_mul(u3d[:, :].rearrange("p (c n) -> p c n", n=J),
                     d3n[:, :].rearrange("p (c n) -> p c n", n=J),
                     blk_t[:, CONF3].unsqueeze(1).broadcast_to([P, 3, J]))
        kscr3 = S([P, 72], "kscr3")
        A.activation(kscr3[:, :], u3d[:, :], AF.Abs,
                     accum_out=comp[0:P, 1:2])

        # pose/betas subs (Pool) + Square-accumulate (ACT); host pre-masked
        dp = S([P, 216], "dp", bf16)
        G.tensor_sub(dp[:, :], blk_t[:, RP], blk_t[:, RG])
        pscr2 = S([P, 216], "pscr2", bf16)
        A.activation(pscr2[:, :], dp[:, :], AF.Square,
                     accum_out=comp[0:P, 3:4])
        db = S([P, 10], "db", bf16)
        G.tensor_sub(db[:, :], blk_t[:, PB], blk_t[:, GS])
        bscr = S([P, 10], "bscr", bf16)
        A.activation(bscr[:, :], db[:, :], AF.Square,
                     accum_out=comp[0:P, 4:5])

        # r = clamp(z/(2 p^3))
        pinv = S([P, 1], "pinv")
        V.reciprocal(pinv[:, :], pp[:, :])
        pv2 = S([P, 1], "pv2")
        V.tensor_mul(pv2[:, :], pinv[:, :], pinv[:, :])
        pv3h = S([P, 1], "pv3h")   # 0.5 * p^-3
        V.scalar_tensor_tensor(pv3h[:, :], pv2[:, :], 0.5, pinv[:, :],
                               OP.mult, OP.mult)
        r1 = S([P, 1], "r1")
        V.scalar_tensor_tensor(r1[:, :], zz[:, :], pv3h[:, :],
                               cst_t[:, RCL], OP.mult, OP.min)

        # powers of r: pw = [r, r^2, ..., r^9]
        pw = S([P, 9], "pw")
        V.tensor_single_scalar(pw[:, 0:1], r1[:, :], -RCLAMP, OP.max)
        V.tensor_mul(pw[:, 1:2], pw[:, 0:1], pw[:, 0:1])
        V.tensor_scalar_mul(pw[:, 2:4], pw[:, 0:2], pw[:, 1:2])
        V.tensor_scalar_mul(pw[:, 4:8], pw[:, 0:4], pw[:, 3:4])
        V.tensor_mul(pw[:, 8:9], pw[:, 3:4], pw[:, 4:5])
        # both outer-root polynomials from one mul + one reduce
        pprod = S([P, 18], "pprod")
        V.tensor_mul(pprod[:, :].rearrange("p (g t) -> p g t", g=2),
                     cst_t[:, PC].rearrange("p (g t) -> p g t", g=2),
                     pw[:, :].unsqueeze(1).broadcast_to([P, 2, 9]))
        xr = S([P, 2], "xr")
        V.tensor_reduce(xr[:, :], pprod[:, :].rearrange(
            "p (g t) -> p g t", g=2), axis=AX.X, op=OP.add)
        x = S([P, 2], "xroots")
        V.tensor_add(x[:, :], xr[:, :], cst_t[:, C0])

        # rzt here: Pool's pzt is ready by now, so DVE never stalls on it
        rzt = S([P, J], "rzt")
        V.reciprocal(rzt[:, :], pzt[:, :])
        aa = S([P, 48], "aa")
        G.tensor_mul(aa[:, :].rearrange("p (c n) -> p c n", n=J),
                     pxy[:, :].rearrange("p (c n) -> p c n", n=J),
                     rzt[:, :].unsqueeze(1).broadcast_to([P, 2, J]))
        dkp = S([P, 48], "dkp")
        G.tensor_sub(dkp[:, :], aa[:, :], blk_t[:, G2])
        u2d = S([P, 48], "u2d")
        G.tensor_mul(u2d[:, :].rearrange("p (c n) -> p c n", n=J),
                     dkp[:, :].rearrange("p (c n) -> p c n", n=J),
                     blk_t[:, CONF2].unsqueeze(1).broadcast_to([P, 2, J]))
        kscr = S([P, 48], "kscr")
        A.activation(kscr[:, :], u2d[:, :], AF.Abs,
                     accum_out=comp[0:P, 0:1])

        # eigenvalues: lam = [l1, lmid, l3=detA/(l1*lmid)], clamped >= TINY
        lamt = S([P, 3], "lamt")
        V.scalar_tensor_tensor(lamt[:, 0:3:2], x[:, :], tp[:, :],
                               q3rd[:, :].broadcast_to([P, 2]),
                               OP.mult, OP.add)
        t13 = S([P, 1], "t13")
        V.tensor_add(t13[:, :], lamt[:, 0:1], lamt[:, 2:3])
        V.tensor_sub(lamt[:, 1:2], qsum[:, :], t13[:, :])
        t12 = S([P, 1], "t12")
        V.tensor_mul(t12[:, :], lamt[:, 0:1], lamt[:, 1:2])
        t12g = S([P, 1], "t12g")
        V.tensor_single_scalar(t12g[:, :], t12[:, :], TINY, OP.max)
        rt12 = S([P, 1], "rt12")
        V.reciprocal(rt12[:, :], t12g[:, :])
        V.tensor_mul(lamt[:, 2:3], detA[:, :], rt12[:, :])
        lam = S([P, 3], "lam")
        V.tensor_single_scalar(lam[:, :], lamt[:, :], TINY, OP.max)
        s3t = S([P, 3], "s3t")
        A.activation(s3t[:, :], lam[:, :], AF.Sqrt)

        # fill the sqrt wait: gap products + scale coefficient
        v1i = S([P, 1], "v1i")
        V.reciprocal(v1i[:, :], var1[:, :])
        cpre = S([P, 1], "cpre")   # pv2 * v1i / 3
        V.scalar_tensor_tensor(cpre[:, :], pv2[:, :], 1.0 / 3.0,
                               v1i[:, :], OP.mult, OP.mult)
        gA = S([P, 2], "gA")   # [l1-lmid, lmid-l3]
        V.tensor_sub(gA[:, :], lam[:, 0:2], lam[:, 1:3])
        g02 = S([P, 1], "g02")
        V.tensor_add(g02[:, :], gA[:, 0:1], gA[:, 1:2])
        Dt = S([P, 3], "Dt")   # signed gap products
        V.tensor_mul(Dt[:, 0:1], gA[:, 0:1], g02[:, :])
        V.scalar_tensor_tensor(Dt[:, 1:2], gA[:, 0:1], -1.0, gA[:, 1:2],
                               OP.mult, OP.mult)
        V.tensor_mul(Dt[:, 2:3], g02[:, :], gA[:, 1:2])
        rD = S([P, 3], "rD")
        V.reciprocal(rD[:, :], Dt[:, :])

        # scl = (s1+s2+sgn*s3) * cpre  (post-sqrt: 4 small ops)
        sinv = S([P, 3], "sinv")
        V.reciprocal(sinv[:, :], s3t[:, :])
        s2s = S([P, 1], "s2s")
        V.tensor_mul(s2s[:, :], s3t[:, 2:3], sgn[:, :])
        s01 = S([P, 1], "s01")
        V.tensor_add(s01[:, :], s3t[:, 0:1], s3t[:, 1:2])
        ssum = S([P, 1], "ssum")
        V.tensor_add(ssum[:, :], s01[:, :], s2s[:, :])
        scl = S([P, 1], "scl")
        V.tensor_mul(scl[:, :], ssum[:, :], cpre[:, :])

        # mm9: [m | m*lam | m*linv] -> one reduce gives (al2, t1, t0)
        linv = S([P, 3], "linv")
        V.tensor_mul(linv[:, :], sinv[:, :], sinv[:, :])
        mm9 = S([P, 9], "mm9")
        V.tensor_mul(mm9[:, 0:3], rD[:, :], sinv[:, :])
        V.tensor_mul(mm9[:, 2:3], mm9[:, 2:3], sgn[:, :])
        V.tensor_mul(mm9[:, 3:6], mm9[:, 0:3], lam[:, :])
        V.tensor_mul(mm9[:, 6:9], mm9[:, 0:3], linv[:, :])
        asum = S([P, 3], "asum")
        V.tensor_reduce(asum[:, :], mm9[:, :].rearrange(
            "p (g i) -> p g i", g=3), axis=AX.X, op=OP.add)
        al1 = S([P, 1], "al1")
        V.scalar_tensor_tensor(al1[:, :], asum[:, 0:1], nqsum[:, :],
                               asum[:, 1:2], OP.mult, OP.add)
        al0 = S([P, 1], "al0")
        V.tensor_mul(al0[:, :], asum[:, 2:3], detA[:, :])

        aI = S([P, 9], "aI")
        V.tensor_scalar_mul(aI[:, :], eye9, al0[:, :])
        W1 = S([P, 9], "W1")
        V.scalar_tensor_tensor(W1[:, :], A29[:, :], asum[:, 0:1], aI[:, :],
                               OP.mult, OP.add)
        W9 = S([P, 9], "W9")
        V.scalar_tensor_tensor(W9[:, :], A9[:, :], al1[:, :], W1[:, :],
                               OP.mult, OP.add)

        # R = W K^T ; RX1 ; Y ; d2
        rq = S([P, 27], "rq")
        V.tensor_mul(
            rq[:, :].rearrange("p (a b c) -> p a b c", a=3, b=3),
            W9[:, :].rearrange("p (a c) -> p a c", a=3)
                .unsqueeze(2).broadcast_to([P, 3, 3, 3]),
            K9[:, :].rearrange("p (b c) -> p b c", b=3)
                .unsqueeze(1).broadcast_to([P, 3, 3, 3]))
        R9 = S([P, 9], "R9")
        V.tensor_reduce(R9[:, :], rq[:, :].rearrange(
            "p (a b c) -> p a b c", a=3, b=3), axis=AX.X, op=OP.add)
        rxq = S([P, 216], "rxq")
        V.tensor_mul(
            rxq[:, :].rearrange("p (i n j) -> p i n j", i=3, n=J),
            R9[:, :].rearrange("p (i j) -> p i j", i=3)
                .unsqueeze(2).broadcast_to([P, 3, J, 3]),
            X1n.rearrange("p (j n) -> p n j", j=3)
                .unsqueeze(1).broadcast_to([P, 3, J, 3]))
        rx1 = S([P, 72], "rx1")
        V.tensor_reduce(rx1[:, :].rearrange("p (i n) -> p i n", i=3),
                        rxq[:, :].rearrange("p (i n j) -> p i n j",
                                            i=3, n=J),
                        axis=AX.X, op=OP.add)
        Yt = S([P, 72], "Yt")
        V.scalar_tensor_tensor(Yt[:, :], rx1[:, :], scl[:, :], X2n,
                               OP.mult, OP.subtract)
        Y2 = S([P, 72], "Y2")
        V.tensor_mul(Y2[:, :], Yt[:, :], Yt[:, :])
        d2 = S([P, J], "d2")
        V.tensor_reduce(d2[:, :],
                        Y2[:, :].rearrange("p (i n) -> p n i", i=3),
                        axis=AX.X, op=OP.add)
        dscr = S([P, J], "dscr")
        A.activation(dscr[:, :], d2[:, :], AF.Sqrt,
                     accum_out=comp[0:P, 5:6])

        # ---------------- vertex L1: 2*sum(max) - sum(a+b) ------------------
        # DVE: one tensor_tensor_reduce per chunk, accumulator chained.
        accs = []
        if USE_TTR:
            vacc = S([128, len(W_CHUNKS)], "vacc")
            for c, w in enumerate(W_CHUNKS):
                vt = vab_ts[c]
                mx = vp.tile([128, w], bf16, name=f"mx{c}", tag="mx")
                V.scalar_tensor_tensor(
                    mx[:, :], vt[:, 0:w], 0.0, vt[:, w:2 * w],
                    OP.bypass, OP.max, accum_out=vacc[:, c:c + 1])
            V.tensor_reduce(comp[:, 2:3], vacc[:, :], axis=AX.X, op=OP.add)
        else:
            vacc = S([128, len(W_CHUNKS)], "vaccf")
            for c, w in enumerate(W_CHUNKS):
                vt = vab_ts[c]
                mx = vp.tile([128, w], bf16, name=f"mx{c}", tag="mx")
                V.scalar_tensor_tensor(mx[:, :], vt[:, 0:w], 1.0,
                                       vt[:, w:2 * w], OP.mult, OP.max)
                sx = vp.tile([128, w], bf16, name=f"sx{c}", tag="sx")
                A.activation(sx[:, :], mx[:, :], AF.Abs, scale=2.0,
                             accum_out=vacc[:, c:c + 1])
            V.tensor_reduce(comp[:, 2:3], vacc[:, :], axis=AX.X, op=OP.add)
        # TensorE: column sums of every 512-wide block -> one PSUM bank
        if USE_MM:
            n_blocks = 2 * F_PACK // 512
            bi = 0
            for c, w in enumerate(W_CHUNKS):
                vt = vab_ts[c]
                for b in range(2 * w // 512):
                    T.matmul(psacc[:, :], ones[:, :],
                             vt[:, 512 * b:512 * (b + 1)],
                             start=(bi == 0), stop=(bi == n_blocks - 1))
                    bi += 1
            sab_scr = S([1, 512], "sab_scr")
            A.activation(sab_scr[:, :], psacc[:, :], AF.Copy,
                         accum_out=comp[0:1, 6:7])
            if USE_WARM_MM:
                V.tensor_single_scalar(comp[0:1, 7:8], pswarm[0:1, 0:1],
                                       0.0, OP.mult)
        else:
            G.memset(comp[0:1, 6:7], 0.0)

        # ---------------- output (Scalar queue) -----------------------------
        A.dma_start(out_d[:, :], comp[:, :])

    nc.compile()
    return nc


_PROGRAM = None


def _get_program():
    global _PROGRAM
    if _PROGRAM is None:
        _PROGRAM = build_program()
    return _PROGRAM


def make_in_maps(inputs: dict) -> list:
    import ml_dtypes

    pj = np.asarray(inputs["pred_joints"], np.float32)
    cam = np.asarray(inputs["pred_camera"], np.float32)
    g2 = np.asarray(inputs["gt_keypoints_2d"], np.float32)
    g3 = np.asarray(inputs["gt_keypoints_3d"], np.float32)
    rp = np.asarray(inputs["pred_rotmat"], np.float32).reshape(512, 216)
    rg = np.asarray(inputs["gt_rotmat"], np.float32).reshape(512, 216)
    pb = np.asarray(inputs["pred_betas"], np.float32)
    gs = np.asarray(inputs["gt_shape"], np.float32)
    hs = np.asarray(inputs["has_smpl"], np.int32)
    va = np.asarray(inputs["pred_vertices"], np.float32).reshape(512, VERT_F)
    vb = np.asarray(inputs["gt_vertices"], np.float32).reshape(512, VERT_F)
    cst = _cst_array()

    idx = np.nonzero(hs > 0)[0]
    assert idx.size <= N_CORES * PACK_CAP, (
        f"n_valid={idx.size} exceeds vertex pack capacity")

    def packed(src, sel):
        buf = np.zeros(128 * F_PACK, ml_dtypes.bfloat16)
        if sel.size:
            flat = src[sel].reshape(-1)
            buf[:flat.size] = flat.astype(ml_dtypes.bfloat16)
        return buf.reshape(128, F_PACK)

    in_maps = []
    for c in range(N_CORES):
        sl = slice(B_PER_CORE * c, B_PER_CORE * (c + 1))
        sel = idx[c::N_CORES]
        mask = (hs[sl] > 0).astype(np.float32)[:, None]
        blk = np.empty((B_PER_CORE, BLK_COLS), np.float32)
        blk[:, 0:72] = pj[sl].transpose(0, 2, 1).reshape(B_PER_CORE, 72)
        blk[:, 72:144] = g3[sl, :, :3].transpose(0, 2, 1).reshape(
            B_PER_CORE, 72)
        blk[:, CONF3] = g3[sl, :, 3]
        blk[:, CAM] = cam[sl]
        blk[:, G2] = ((g2[sl, :, :2] - 256.0) / 1000.0).transpose(
            0, 2, 1).reshape(B_PER_CORE, 48)
        blk[:, CONF2] = g2[sl, :, 2] * 1000.0
        blk[:, RP] = rp[sl] * mask
        blk[:, RG] = rg[sl] * mask
        blk[:, PB] = pb[sl] * mask
        blk[:, GS] = gs[sl] * mask
        va_p = packed(va, sel)
        vb_p = packed(vb, sel)
        vab = np.empty((128, 2 * F_PACK), ml_dtypes.bfloat16)
        off = 0
        for w in W_CHUNKS:
            vab[:, 2 * off:2 * off + w] = va_p[:, off:off + w]
            vab[:, 2 * off + w:2 * off + 2 * w] = vb_p[:, off:off + w]
            off += w
        in_maps.append({
            "cst": np.ascontiguousarray(cst, np.float32),
            "blk": np.ascontiguousarray(blk.astype(ml_dtypes.bfloat16)),
            "vab": np.ascontiguousarray(vab),
        })
    return in_maps


def combine_partials(parts: np.ndarray, n_valid: float) -> np.float32:
    # parts: [n_cores, 128, 8]
    p64 = parts.astype(np.float64)
    kp2d = p64[:, 0:B_PER_CORE, 0].sum()
    kp3d = p64[:, 0:B_PER_CORE, 1].sum()
    vmax2 = p64[:, :, 2].sum()
    pose = p64[:, 0:B_PER_CORE, 3].sum()
    betas = p64[:, 0:B_PER_CORE, 4].sum()
    pa = p64[:, 0:B_PER_CORE, 5].sum()
    sab = p64[:, 0, 6].sum()
    vert = 2.0 * vmax2 - sab   # comp col2 holds sum(max(a,b))
    B = 512.0
    total = (4.0 * kp2d / (512.0 * B * J * 2)
             + 4.0 * kp3d / (B * J * 3)
             + vert / (n_valid * VERT_F + EPS)
             + pose / (n_valid * 216 + EPS)
             + 0.01 * betas / (n_valid * 10 + EPS)
             + pa / (B * J))
    return np.float32(total)


def kernel(**inputs) -> np.ndarray:
    nc = _get_program()
    in_maps = make_in_maps(inputs)
    res = run_bass_kernel_spmd(nc, in_maps, core_ids=list(range(N_CORES)))
    parts = np.stack([res.results[c]["out"] for c in range(N_CORES)])
    nv = float((np.asarray(inputs["has_smpl"]) > 0).sum())
    return np.asarray(combine_partials(parts, nv))


# revision 13
# speedup vs baseline: 1.0931x; 1.0931x over previous
"""Trainium2 Bass kernel for the BMP loss (nn_BMPLoss_24670292148307). V4b.

Data-parallel over 8 NeuronCores; host combines per-core partial sums.

V4b vs the 34us V3 baseline (critical-path + queue restructuring; vertex
math identical to V3: DVE add of pre-negated gt + ACT Abs-accumulate):
  - va/(-gt) ship interleaved per chunk in ONE dram tensor: one DMA
    trigger per chunk (DIRECT2D costs ~0.6us of sequencer time each),
    all on the SP HWDGE queue; blk/cst/out ride the ACT queue.  4 chunks
    sized [512,1792,1792,1536] -- the small first chunk starts the ACT
    Abs pipeline ~2us earlier and the add/abs of each chunk overlaps the
    DMA of the next.
  - Chebyshev outer-root evaluation in power basis: build [r..r^9] in 4
    DVE ops + one mul + one reduce (~0.5us serial) instead of the
    10-step Horner chain (2.1us serial).
  - A^2 computed right after A (off the critical path); det(K) branch
    fills the p=sqrt() ACT wait; the scale coefficient (pv2*v1i/3) is
    precomputed before the s=sqrt(lam) wait so only 4 small ops remain
    after it.
  - pose/betas/shape masks folded into the host pack (rows pre-zeroed);
    the mask multiplies and the comp memset are dropped (host sums only
    the rows each accumulator writes).
  - trailing probe op (comp col 7, ignored by host): tensor_scalar
    abs_max with accum_out on a full chunk, to read its perf mode from
    the trace.
"""
import numpy as np
from contextlib import ExitStack

import concourse.bass as bass
import concourse.bacc as bacc
import concourse.tile as tile
import concourse.mybir as mybir
from concourse.bass_utils import run_bass_kernel_spmd

f32 = mybir.dt.float32
bf16 = mybir.dt.bfloat16
AF = mybir.ActivationFunctionType
OP = mybir.AluOpType
AX = mybir.AxisListType

B_PER_CORE = 64
N_CORES = 8
J = 24
VERT_F = 20670           # real floats per sample (6890*3)
F_PACK = 5632            # padded stream cols per tensor
PACK_CAP = 34            # vertex slots per core (128*5632/20670 = 34.8)
W_CHUNKS = [512, 1792, 1792, 1536]
EPS = 1e-8
TINY = 1e-30
RCLAMP = 0.99995

# blk (bf16) column map
PG6 = slice(0, 144)      # (c,n): rows 0-2 pj xyz, 3-5 gt3 xyz, joint-minor
CONF3 = slice(144, 168)
CAM = slice(168, 171)
G2 = slice(171, 219)     # (c,n), pre-shifted by -256, /1000
CONF2 = slice(219, 243)
RP = slice(243, 459)     # pre-masked by has_smpl
RG = slice(459, 675)     # pre-masked
PB = slice(675, 685)     # pre-masked
GS = slice(685, 695)     # pre-masked
BLK_COLS = 695

# cst (f32) column map
PC = slice(0, 18)        # cols 0-8 = P1C[1..9], 9-17 = P3C[1..9]
C0 = slice(18, 20)       # [P1C[0], P3C[0]]
EYE9 = slice(20, 29)
EYE3 = slice(29, 38)     # eye/3
RCL = slice(38, 39)      # RCLAMP
CST_COLS = 40

P1C = [0.8649274597522203, 0.17578197434414333, -0.002087134697444787,
       -0.1271791091353304, -0.3070988770461487, 0.6789215326112841,
       0.5727490378285598, -1.068537975408937, -0.3683220235409602,
       0.5818562170395759]
P3C = [-0.8649274597522203, 0.17578197434414353, 0.002087134697442622,
       -0.1271791091353331, 0.3070988770461617, 0.6789215326112932,
       -0.5727490378285826, -1.068537975408948, 0.3683220235409723,
       0.58185621703958]


def _cst_array() -> np.ndarray:
    c = np.zeros((B_PER_CORE, CST_COLS), np.float32)
    for t in range(9):
        c[:, t] = np.float32(P1C[t + 1])
        c[:, 9 + t] = np.float32(P3C[t + 1])
    c[:, 18] = np.float32(P1C[0])
    c[:, 19] = np.float32(P3C[0])
    eye = np.eye(3, dtype=np.float32).reshape(9)
    c[:, EYE9] = eye
    c[:, EYE3] = eye / 3.0
    c[:, RCL] = RCLAMP
    return c


def build_program():
    nc = bacc.Bacc("TRN2", target_bir_lowering=False, debug=False,
                   num_devices=N_CORES)
    P = B_PER_CORE

    cst_d = nc.dram_tensor("cst", [P, CST_COLS], f32, kind="ExternalInput")
    blk_d = nc.dram_tensor("blk", [P, BLK_COLS], bf16, kind="ExternalInput")
    vab_d = nc.dram_tensor("vab", [128, 2 * F_PACK], bf16,
                           kind="ExternalInput")
    out_d = nc.dram_tensor("out", [128, 8], f32, kind="ExternalOutput")

    with tile.TileContext(nc) as tc, ExitStack() as ctx:
        V = nc.vector
        A = nc.scalar
        G = nc.gpsimd
        SP = nc.sync
        sg = ctx.enter_context(tc.tile_pool(name="singles", bufs=1))
        vp = ctx.enter_context(tc.tile_pool(name="vp", bufs=2))

        def S(shape, name, dtype=f32):
            return sg.tile(list(shape), dtype, name=name)

        comp = S([128, 8], "comp")

        # first ACT op is a Sqrt so the table loader picks the sqrt set once
        warm = S([1, 1], "warm")
        G.memset(warm[:, :], 1.0)
        warm2 = S([1, 1], "warm2")
        A.activation(warm2[:, :], warm[:, :], AF.Sqrt)

        # ---------------- input DMAs ----------------------------------------
        blk_t = S([P, BLK_COLS], "blk_t", bf16)
        A.dma_start(blk_t[:, :], blk_d[:, :])
        cst_t = S([P, CST_COLS], "cst_t")
        A.dma_start(cst_t[:, :], cst_d[:, :])
        vab_ts = []
        off = 0
        for c, w in enumerate(W_CHUNKS):
            vt = sg.tile([128, 2 * w], bf16, name=f"vab{c}")
            SP.dma_start(vt[:, :], vab_d[:, 2 * off:2 * off + 2 * w])
            vab_ts.append(vt)
            off += w

        pg6 = blk_t[:, PG6]
        eye9 = cst_t[:, EYE9]
        eye3 = cst_t[:, EYE3]
        t1 = S([P, 1], "t1")

        # ================ Procrustes chain (DVE) ================
        musum = S([P, 6], "musum")
        V.tensor_reduce(musum[:, :], pg6.rearrange("p (c n) -> p c n", n=J),
                        axis=AX.X, op=OP.add)
        Xn = S([P, 144], "Xn")     # (musum/24 - pg6): negated centered coords
        V.scalar_tensor_tensor(
            Xn[:, :].rearrange("p (c n) -> p c n", n=J),
            musum[:, :].unsqueeze(2).broadcast_to([P, 6, J]), 1.0 / J,
            pg6.rearrange("p (c n) -> p c n", n=J), OP.mult, OP.subtract)
        X1n = Xn[:, 0:72]
        X2n = Xn[:, 72:144]
        var1 = S([P, 1], "var1")
        vscr = S([P, 72], "vscr")
        A.activation(vscr[:, :], X1n, AF.Square, accum_out=var1[:, :])
        V.tensor_scalar(t1[:, :], blk_t[:, CAM][:, 0:1], 512.0, EPS,
                        OP.mult, OP.add)
        rt1 = S([P, 1], "rt1")
        V.reciprocal(rt1[:, :], t1[:, :])

        # kp2d prep front-loaded on Pool so rzt slots into the chain early
        depth = S([P, 1], "depth")
        G.tensor_single_scalar(depth[:, :], rt1[:, :], 2000.0, OP.mult)
        pxy = S([P, 48], "pxy", bf16)
        G.tensor_add(pxy[:, :].rearrange("p (c n) -> p c n", n=J),
                     blk_t[:, PG6].rearrange("p (c n) -> p c n", n=J)[:, 0:2],
                     blk_t[:, CAM][:, 1:3].unsqueeze(2).broadcast_to([P, 2, J]))
        pzt = S([P, J], "pzt")
        G.tensor_add(pzt[:, :], blk_t[:, 48:72],
                     depth[:, :].broadcast_to([P, J]))

        # K = X1 X2^T
        kq = S([P, 216], "kq")
        V.tensor_mul(
            kq[:, :].rearrange("p (i j n) -> p i j n", i=3, j=3),
            X1n.rearrange("p (i n) -> p i n", i=3)
                .unsqueeze(2).broadcast_to([P, 3, 3, J]),
            X2n.rearrange("p (j n) -> p j n", j=3)
                .unsqueeze(1).broadcast_to([P, 3, 3, J]))
        K9 = S([P, 9], "K9")
        V.tensor_reduce(K9[:, :], kq[:, :].rearrange(
            "p (i j n) -> p i j n", i=3, j=3), axis=AX.X, op=OP.add)

        # det(K) on DVE (feeds detA for r, and the sign)
        dQ = S([P, 9], "dQ")
        V.tensor_mul(
            dQ[:, :].rearrange("p (a b) -> p a b", a=3),
            K9[:, 3:6].unsqueeze(2).broadcast_to([P, 3, 3]),
            K9[:, 6:9].unsqueeze(1).broadcast_to([P, 3, 3]))
        dD = S([P, 9], "dD")
        V.tensor_sub(dD[:, :].rearrange("p (a b) -> p a b", a=3),
                     dQ[:, :].rearrange("p (a b) -> p a b", a=3),
                     dQ[:, :].rearrange("p (b a) -> p a b", b=3))
        du1 = S([P, 2], "du1")
        V.tensor_mul(du1[:, :], K9[:, 0:2], dD[:, 5:7])
        du2 = S([P, 1], "du2")
        V.tensor_mul(du2[:, :], K9[:, 2:3], dD[:, 1:2])
        du1r = S([P, 1], "du1r")
        V.tensor_reduce(du1r[:, :], du1[:, :], axis=AX.X, op=OP.add)
        detK = S([P, 1], "detK")
        V.tensor_add(detK[:, :], du1r[:, :], du2[:, :])
        detA = S([P, 1], "detA")
        V.tensor_mul(detA[:, :], detK[:, :], detK[:, :])
        sg0 = S([P, 1], "sg0")
        V.tensor_single_scalar(sg0[:, :], detK[:, :], 0.0, OP.is_ge)
        sgn = S([P, 1], "sgn")
        V.tensor_scalar(sgn[:, :], sg0[:, :], 2.0, -1.0, OP.mult, OP.add)

        # A = K^T K
        aq = S([P, 27], "aq")
        V.tensor_mul(
            aq[:, :].rearrange("p (i j k) -> p i j k", i=3, j=3),
            K9[:, :].rearrange("p (k i) -> p i k", k=3)
                .unsqueeze(2).broadcast_to([P, 3, 3, 3]),
            K9[:, :].rearrange("p (k j) -> p j k", k=3)
                .unsqueeze(1).broadcast_to([P, 3, 3, 3]))
        A9 = S([P, 9], "A9")
        V.tensor_reduce(A9[:, :], aq[:, :].rearrange(
            "p (i j k) -> p i j k", i=3, j=3), axis=AX.X, op=OP.add)
        # A^2 right after A: needed only at W, but free to do now
        a2q = S([P, 27], "a2q")
        V.tensor_mul(
            a2q[:, :].rearrange("p (i j k) -> p i j k", i=3, j=3),
            A9[:, :].rearrange("p (i k) -> p i k", i=3)
                .unsqueeze(2).broadcast_to([P, 3, 3, 3]),
            A9[:, :].rearrange("p (k j) -> p j k", k=3)
                .unsqueeze(1).broadcast_to([P, 3, 3, 3]))
        A29 = S([P, 9], "A29")
        V.tensor_reduce(A29[:, :], a2q[:, :].rearrange(
            "p (i j k) -> p i j k", i=3, j=3), axis=AX.X, op=OP.add)

        qsum = S([P, 1], "qsum")
        V.tensor_reduce(qsum[:, :], A9[:, 0:9:4], axis=AX.X, op=OP.add)
        q3rd = S([P, 1], "q3rd")
        V.tensor_single_scalar(q3rd[:, :], qsum[:, :], 1.0 / 3.0, OP.mult)
        q2 = S([P, 1], "q2")
        V.tensor_mul(q2[:, :], q3rd[:, :], q3rd[:, :])
        q3 = S([P, 1], "q3")
        V.tensor_mul(q3[:, :], q2[:, :], q3rd[:, :])
        nqsum = S([P, 1], "nqsum")
        V.tensor_single_scalar(nqsum[:, :], qsum[:, :], -1.0, OP.mult)
        aqn = S([P, 9], "aqn")
        V.scalar_tensor_tensor(aqn[:, :], eye3, qsum[:, :], A9[:, :],
                               OP.mult, OP.subtract)
        pscr = S([P, 9], "pscr")
        V.tensor_mul(pscr[:, :], aqn[:, :], aqn[:, :])
        p2r = S([P, 1], "p2r")
        V.tensor_reduce(p2r[:, :], pscr[:, :], axis=AX.X, op=OP.add)
        p2g = S([P, 1], "p2g")
        V.tensor_scalar(p2g[:, :], p2r[:, :], 1.0 / 6.0, TINY,
                        OP.mult, OP.max)
        pp = S([P, 1], "pp")
        A.activation(pp[:, :], p2g[:, :], AF.Sqrt)
        tp = S([P, 1], "tp")
        G.tensor_single_scalar(tp[:, :], pp[:, :], 2.0, OP.mult)

        # z = detA - q^3 + 3 q p^2 (fills the sqrt wait)
        zu = S([P, 1], "zu")
        V.tensor_mul(zu[:, :], q3rd[:, :], p2g[:, :])
        zv = S([P, 1], "zv")
        V.scalar_tensor_tensor(zv[:, :], zu[:, :], 3.0, q3[:, :],
                               OP.mult, OP.subtract)
        zz = S([P, 1], "zz")
        V.tensor_add(zz[:, :], detA[:, :], zv[:, :])

        # ---------------- kp3d (Pool prep, ACT accumulate) ------------------
        pd = S([P, 72], "pd", bf16)
        G.tensor_sub(pd[:, :], blk_t[:, 0:72], blk_t[:, 72:144])
        pdr = pd[:, :].rearrange("p (c n) -> p c n", n=J)
        pel = S([P, 3], "pel", bf16)
        G.tensor_add(pel[:, :], pdr[:, :, 2].squeeze(), pdr[:, :, 3].squeeze())
        pel2 = S([P, 3], "pel2", bf16)
        G.tensor_single_scalar(pel2[:, :], pel[:, :], 0.5, OP.mult)
        d3n = S([P, 72], "d3n", bf16)
        G.tensor_sub(d3n[:, :].rearrange("p (c n) -> p c n", n=J),
                     pdr, pel2[:, :].unsqueeze(2).broadcast_to([P, 3, J]))
        u3d = S([P, 72], "u3d", bf16)
        G.tensor_mul(u3d[:, :].rearrange("p (c n) -> p c n", n=J),
                     d3n[:, :].rearrange("p (c n) -> p c n", n=J),
                     blk_t[:, CONF3].unsqueeze(1).broadcast_to([P, 3, J]))
        kscr3 = S([P, 72], "kscr3")
        A.activation(kscr3[:, :], u3d[:, :], AF.Abs,
                     accum_out=comp[0:P, 1:2])

        # pose/betas subs (Pool) + Square-accumulate (ACT); host pre-masked
        dp = S([P, 216], "dp", bf16)
        G.tensor_sub(dp[:, :], blk_t[:, RP], blk_t[:, RG])
        pscr2 = S([P, 216], "pscr2", bf16)
        A.activation(pscr2[:, :], dp[:, :], AF.Square,
                     accum_out=comp[0:P, 3:4])
        db = S([P, 10], "db", bf16)
        G.tensor_sub(db[:, :], blk_t[:, PB], blk_t[:, GS])
        bscr = S([P, 10], "bscr", bf16)
        A.activation(bscr[:, :], db[:, :], AF.Square,
                     accum_out=comp[0:P, 4:5])

        # r = clamp(z/(2 p^3))
        pinv = S([P, 1], "pinv")
        V.reciprocal(pinv[:, :], pp[:, :])
        pv2 = S([P, 1], "pv2")
        V.tensor_mul(pv2[:, :], pinv[:, :], pinv[:, :])
        pv3h = S([P, 1], "pv3h")   # 0.5 * p^-3
        V.scalar_tensor_tensor(pv3h[:, :], pv2[:, :], 0.5, pinv[:, :],
                               OP.mult, OP.mult)
        r1 = S([P, 1], "r1")
        V.scalar_tensor_tensor(r1[:, :], zz[:, :], pv3h[:, :],
                               cst_t[:, RCL], OP.mult, OP.min)

        # powers of r: pw = [r, r^2, ..., r^9]
        pw = S([P, 9], "pw")
        V.tensor_single_scalar(pw[:, 0:1], r1[:, :], -RCLAMP, OP.max)
        V.tensor_mul(pw[:, 1:2], pw[:, 0:1], pw[:, 0:1])
        V.tensor_scalar_mul(pw[:, 2:4], pw[:, 0:2], pw[:, 1:2])
        V.tensor_scalar_mul(pw[:, 4:8], pw[:, 0:4], pw[:, 3:4])
        V.tensor_mul(pw[:, 8:9], pw[:, 3:4], pw[:, 4:5])
        # both outer-root polynomials from one mul + one reduce
        pprod = S([P, 18], "pprod")
        V.tensor_mul(pprod[:, :].rearrange("p (g t) -> p g t", g=2),
                     cst_t[:, PC].rearrange("p (g t) -> p g t", g=2),
                     pw[:, :].unsqueeze(1).broadcast_to([P, 2, 9]))
        xr = S([P, 2], "xr")
        V.tensor_reduce(xr[:, :], pprod[:, :].rearrange(
            "p (g t) -> p g t", g=2), axis=AX.X, op=OP.add)
        x = S([P, 2], "xroots")
        V.tensor_add(x[:, :], xr[:, :], cst_t[:, C0])

        # rzt here: Pool's pzt is ready by now, so DVE never stalls on it
        rzt = S([P, J], "rzt")
        V.reciprocal(rzt[:, :], pzt[:, :])
        aa = S([P, 48], "aa")
        G.tensor_mul(aa[:, :].rearrange("p (c n) -> p c n", n=J),
                     pxy[:, :].rearrange("p (c n) -> p c n", n=J),
                     rzt[:, :].unsqueeze(1).broadcast_to([P, 2, J]))
        dkp = S([P, 48], "dkp")
        G.tensor_sub(dkp[:, :], aa[:, :], blk_t[:, G2])
        u2d = S([P, 48], "u2d")
        G.tensor_mul(u2d[:, :].rearrange("p (c n) -> p c n", n=J),
                     dkp[:, :].rearrange("p (c n) -> p c n", n=J),
                     blk_t[:, CONF2].unsqueeze(1).broadcast_to([P, 2, J]))
        kscr = S([P, 48], "kscr")
        A.activation(kscr[:, :], u2d[:, :], AF.Abs,
                     accum_out=comp[0:P, 0:1])

        # eigenvalues: lam = [l1, lmid, l3=detA/(l1*lmid)], clamped >= TINY
        lamt = S([P, 3], "lamt")
        V.scalar_tensor_tensor(lamt[:, 0:3:2], x[:, :], tp[:, :],
                               q3rd[:, :].broadcast_to([P, 2]),
                               OP.mult, OP.add)
        t13 = S([P, 1], "t13")
        V.tensor_add(t13[:, :], lamt[:, 0:1], lamt[:, 2:3])
        V.tensor_sub(lamt[:, 1:2], qsum[:, :], t13[:, :])
        t12 = S([P, 1], "t12")
        V.tensor_mul(t12[:, :], lamt[:, 0:1], lamt[:, 1:2])
        t12g = S([P, 1], "t12g")
        V.tensor_single_scalar(t12g[:, :], t12[:, :], TINY, OP.max)
        rt12 = S([P, 1], "rt12")
        V.reciprocal(rt12[:, :], t12g[:, :])
        V.tensor_mul(lamt[:, 2:3], detA[:, :], rt12[:, :])
        lam = S([P, 3], "lam")
        V.tensor_single_scalar(lam[:, :], lamt[:, :], TINY, OP.max)
        s3t = S([P, 3], "s3t")
        A.activation(s3t[:, :], lam[:, :], AF.Sqrt)

        # fill the sqrt wait: gap products + scale coefficient
        v1i = S([P, 1], "v1i")
        V.reciprocal(v1i[:, :], var1[:, :])
        cpre = S([P, 1], "cpre")   # pv2 * v1i / 3
        V.scalar_tensor_tensor(cpre[:, :], pv2[:, :], 1.0 / 3.0,
                               v1i[:, :], OP.mult, OP.mult)
        gA = S([P, 2], "gA")   # [l1-lmid, lmid-l3]
        V.tensor_sub(gA[:, :], lam[:, 0:2], lam[:, 1:3])
        g02 = S([P, 1], "g02")
        V.tensor_add(g02[:, :], gA[:, 0:1], gA[:, 1:2])
        Dt = S([P, 3], "Dt")   # signed gap products
        V.tensor_mul(Dt[:, 0:1], gA[:, 0:1], g02[:, :])
        V.scalar_tensor_tensor(Dt[:, 1:2], gA[:, 0:1], -1.0, gA[:, 1:2],
                               OP.mult, OP.mult)
        V.tensor_mul(Dt[:, 2:3], g02[:, :], gA[:, 1:2])
        rD = S([P, 3], "rD")
        V.reciprocal(rD[:, :], Dt[:, :])

        # scl = (s1+s2+sgn*s3) * cpre  (post-sqrt: 4 small ops)
        sinv = S([P, 3], "sinv")
        V.reciprocal(sinv[:, :], s3t[:, :])
        s2s = S([P, 1], "s2s")
        V.tensor_mul(s2s[:, :], s3t[:, 2:3], sgn[:, :])
        s01 = S([P, 1], "s01")
        V.tensor_add(s01[:, :], s3t[:, 0:1], s3t[:, 1:2])
        ssum = S([P, 1], "ssum")
        V.tensor_add(ssum[:, :], s01[:, :], s2s[:, :])
        scl = S([P, 1], "scl")
        V.tensor_mul(scl[:, :], ssum[:, :], cpre[:, :])

        # mm9: [m | m*lam | m*linv] -> one reduce gives (al2, t1, t0)
        linv = S([P, 3], "linv")
        V.tensor_mul(linv[:, :], sinv[:, :], sinv[:, :])
        mm9 = S([P, 9], "mm9")
        V.tensor_mul(mm9[:, 0:3], rD[:, :], sinv[:, :])
        V.tensor_mul(mm9[:, 2:3], mm9[:, 2:3], sgn[:, :])
        V.tensor_mul(mm9[:, 3:6], mm9[:, 0:3], lam[:, :])
        V.tensor_mul(mm9[:, 6:9], mm9[:, 0:3], linv[:, :])
        asum = S([P, 3], "asum")
        V.tensor_reduce(asum[:, :], mm9[:, :].rearrange(
            "p (g i) -> p g i", g=3), axis=AX.X, op=OP.add)
        al1 = S([P, 1], "al1")
        V.scalar_tensor_tensor(al1[:, :], asum[:, 0:1], nqsum[:, :],
                               asum[:, 1:2], OP.mult, OP.add)
        al0 = S([P, 1], "al0")
        V.tensor_mul(al0[:, :], asum[:, 2:3], detA[:, :])

        aI = S([P, 9], "aI")
        V.tensor_scalar_mul(aI[:, :], eye9, al0[:, :])
        W1 = S([P, 9], "W1")
        V.scalar_tensor_tensor(W1[:, :], A29[:, :], asum[:, 0:1], aI[:, :],
                               OP.mult, OP.add)
        W9 = S([P, 9], "W9")
        V.scalar_tensor_tensor(W9[:, :], A9[:, :], al1[:, :], W1[:, :],
                               OP.mult, OP.add)

        # R = W K^T ; RX1 ; Y ; d2
        rq = S([P, 27], "rq")
        V.tensor_mul(
            rq[:, :].rearrange("p (a b c) -> p a b c", a=3, b=3),
            W9[:, :].rearrange("p (a c) -> p a c", a=3)
                .unsqueeze(2).broadcast_to([P, 3, 3, 3]),
            K9[:, :].rearrange("p (b c) -> p b c", b=3)
                .unsqueeze(1).broadcast_to([P, 3, 3, 3]))
        R9 = S([P, 9], "R9")
        V.tensor_reduce(R9[:, :], rq[:, :].rearrange(
            "p (a b c) -> p a b c", a=3, b=3), axis=AX.X, op=OP.add)
        rxq = S([P, 216], "rxq")
        V.tensor_mul(
            rxq[:, :].rearrange("p (i n j) -> p i n j", i=3, n=J),
            R9[:, :].rearrange("p (i j) -> p i j", i=3)
                .unsqueeze(2).broadcast_to([P, 3, J, 3]),
            X1n.rearrange("p (j n) -> p n j", j=3)
                .unsqueeze(1).broadcast_to([P, 3, J, 3]))
        rx1 = S([P, 72], "rx1")
        V.tensor_reduce(rx1[:, :].rearrange("p (i n) -> p i n", i=3),
                        rxq[:, :].rearrange("p (i n j) -> p i n j",
                                            i=3, n=J),
                        axis=AX.X, op=OP.add)
        Yt = S([P, 72], "Yt")
        V.scalar_tensor_tensor(Yt[:, :], rx1[:, :], scl[:, :], X2n,
                               OP.mult, OP.subtract)
        Y2 = S([P, 72], "Y2")
        V.tensor_mul(Y2[:, :], Yt[:, :], Yt[:, :])
        d2 = S([P, J], "d2")
        V.tensor_reduce(d2[:, :],
                        Y2[:, :].rearrange("p (i n) -> p n i", i=3),
                        axis=AX.X, op=OP.add)
        dscr = S([P, J], "dscr")
        A.activation(dscr[:, :], d2[:, :], AF.Sqrt,
                     accum_out=comp[0:P, 5:6])

        # ---------------- vertex L1 (DVE add + ACT Abs-accumulate) ----------
        vacc = S([128, len(W_CHUNKS)], "vacc")
        for c, w in enumerate(W_CHUNKS):
            vt = vab_ts[c]
            d_t = vp.tile([128, w], bf16, name=f"d{c}", tag="d")
            V.tensor_add(d_t[:, :], vt[:, 0:w], vt[:, w:2 * w])
            s_t = vp.tile([128, w], bf16, name=f"s{c}", tag="s")
            A.activation(s_t[:, :], d_t[:, :], AF.Abs,
                         accum_out=vacc[:, c:c + 1])
        V.tensor_reduce(comp[:, 2:3], vacc[:, :], axis=AX.X, op=OP.add)

        # ---------------- output (Scalar queue) -----------------------------
        A.dma_start(out_d[:, :], comp[:, :])

    nc.compile()
    return nc


_PROGRAM = None


def _get_program():
    global _PROGRAM
    if _PROGRAM is None:
        _PROGRAM = build_program()
    return _PROGRAM


def make_in_maps(inputs: dict) -> list:
    import ml_dtypes

    pj = np.asarray(inputs["pred_joints"], np.float32)
    cam = np.asarray(inputs["pred_camera"], np.float32)
    g2 = np.asarray(inputs["gt_keypoints_2d"], np.float32)
    g3 = np.asarray(inputs["gt_keypoints_3d"], np.float32)
    rp = np.asarray(inputs["pred_rotmat"], np.float32).reshape(512, 216)
    rg = np.asarray(inputs["gt_rotmat"], np.float32).reshape(512, 216)
    pb = np.asarray(inputs["pred_betas"], np.float32)
    gs = np.asarray(inputs["gt_shape"], np.float32)
    hs = np.asarray(inputs["has_smpl"], np.int32)
    va = np.asarray(inputs["pred_vertices"], np.float32).reshape(512, VERT_F)
    vb = np.asarray(inputs["gt_vertices"], np.float32).reshape(512, VERT_F)
    cst = _cst_array()

    idx = np.nonzero(hs > 0)[0]
    assert idx.size <= N_CORES * PACK_CAP, (
        f"n_valid={idx.size} exceeds vertex pack capacity")

    def packed(src, sel, negate):
        buf = np.zeros(128 * F_PACK, ml_dtypes.bfloat16)
        if sel.size:
            flat = src[sel].reshape(-1)
            if negate:
                flat = -flat
            buf[:flat.size] = flat.astype(ml_dtypes.bfloat16)
        return buf.reshape(128, F_PACK)

    in_maps = []
    for c in range(N_CORES):
        sl = slice(B_PER_CORE * c, B_PER_CORE * (c + 1))
        sel = idx[c::N_CORES]
        mask = (hs[sl] > 0).astype(np.float32)[:, None]
        blk = np.empty((B_PER_CORE, BLK_COLS), np.float32)
        blk[:, 0:72] = pj[sl].transpose(0, 2, 1).reshape(B_PER_CORE, 72)
        blk[:, 72:144] = g3[sl, :, :3].transpose(0, 2, 1).reshape(
            B_PER_CORE, 72)
        blk[:, CONF3] = g3[sl, :, 3]
        blk[:, CAM] = cam[sl]
        blk[:, G2] = ((g2[sl, :, :2] - 256.0) / 1000.0).transpose(
            0, 2, 1).reshape(B_PER_CORE, 48)
        blk[:, CONF2] = g2[sl, :, 2] * 1000.0
        blk[:, RP] = rp[sl] * mask
        blk[:, RG] = rg[sl] * mask
        blk[:, PB] = pb[sl] * mask
        blk[:, GS] = gs[sl] * mask
        va_p = packed(va, sel, False)
        vb_p = packed(vb, sel, True)
        vab = np.empty((128, 2 * F_PACK), ml_dtypes.bfloat16)
        off = 0
        for w in W_CHUNKS:
            vab[:, 2 * off:2 * off + w] = va_p[:, off:off + w]
            vab[:, 2 * off + w:2 * off + 2 * w] = vb_p[:, off:off + w]
            off += w
        in_maps.append({
            "cst": np.ascontiguousarray(cst, np.float32),
            "blk": np.ascontiguousarray(blk.astype(ml_dtypes.bfloat16)),
            "vab": np.ascontiguousarray(vab),
        })
    return in_maps


def combine_partials(parts: np.ndarray, n_valid: float) -> np.float32:
    # parts: [n_cores, 128, 8]
    p64 = parts.astype(np.float64)
    kp2d = p64[:, 0:B_PER_CORE, 0].sum()
    kp3d = p64[:, 0:B_PER_CORE, 1].sum()
    vert = p64[:, :, 2].sum()
    pose = p64[:, 0:B_PER_CORE, 3].sum()
    betas = p64[:, 0:B_PER_CORE, 4].sum()
    pa = p64[:, 0:B_PER_CORE, 5].sum()
    B = 512.0
    total = (4.0 * kp2d / (512.0 * B * J * 2)
             + 4.0 * kp3d / (B * J * 3)
             + vert / (n_valid * VERT_F + EPS)
             + pose / (n_valid * 216 + EPS)
             + 0.01 * betas / (n_valid * 10 + EPS)
             + pa / (B * J))
    return np.float32(total)


def kernel(**inputs) -> np.ndarray:
    nc = _get_program()
    in_maps = make_in_maps(inputs)
    res = run_bass_kernel_spmd(nc, in_maps, core_ids=list(range(N_CORES)))
    parts = np.stack([res.results[c]["out"] for c in range(N_CORES)])
    nv = float((np.asarray(inputs["has_smpl"]) > 0).sum())
    return np.asarray(combine_partials(parts, nv))
